# revision 6
# baseline (speedup 1.0000x reference)
"""Binarized MLP (64->2048->1024->512->64->1, B=32768) on 8 trn2 NeuronCores, v2.

Changes vs v1 baseline:
- batch tile 1024 (was 512): psums [128,1024] span 2 banks; each DR weight
  load is shared by two N=512 matmuls (cols 0:512 / 512:1024); epilogues run
  once per m-tile at FD=1024 (halved op count, amortized overhead).
- L1 uses an exact 2-term fp16 split of x (was 3-term bf16): 16 full K=128
  fp16 matmuls per tile, no partial/tile_position matmuls. fp16 residual
  bound 2^-21|x| makes threshold flips ~1e-7 probable.
- L5 uses a single bf16 term for w5 (sigmoid output err ~4e-4 relative).

v3: L1 matmuls+epilogues of tile b+1 are interleaved between L2/L3 m-tiles of
tile b, so L1's epilogue latency hides under L2's long matmul spans instead of
gating the PE; h1/h2 are split into half tiles so each layer can start on the
first half while the second half's epilogues drain; x DMAs for tile 0 are
issued before the large w2t/w3t weight loads.

v6: batch tiles are processed in PAIRS through L2-L5: each weight block
(ldweights) streams 4x512 cols (both tiles' halves) instead of 2x512,
halving PE weight switches (~396 -> ~198); psum pools merged into one
4-buffer rotation; L1 of the next pair interleaves between L2 m-tiles.
"""

import sys

import numpy as np

sys.path.insert(0, "/opt/trn_rl_repo")

import ml_dtypes

import concourse.bass as bass
import concourse.bacc as bacc
import concourse.mybir as mybir
import concourse.tile as tile
from concourse.bass import ts, ds
from concourse.bass_utils import run_bass_kernel_spmd
from concourse.masks import make_identity
from contextlib import ExitStack

N_CORES = 8
B = 32768
BC = B // N_CORES          # 4096 rows per core
BT = 1024                  # batch tile
DIMS = [64, 2048, 1024, 512, 64, 1]
EPS = 1e-5

F32 = mybir.dt.float32
BF16 = mybir.dt.bfloat16
FP16 = mybir.dt.float16
FP8 = mybir.dt.float8e4
DR = mybir.MatmulPerfMode.DoubleRow


def _thr(b, g, be, m, v):
    # (z + b - m) * g/sqrt(v+eps) + be > 0  <=>  z > m - be*sqrt(v+eps)/g - b
    s = np.float64(g) / np.sqrt(np.float64(v) + EPS)
    return np.float64(m) - np.float64(be) / s - np.float64(b)


def _feat_major(a, n_feat):
    ks = n_feat // 128
    return np.ascontiguousarray(a.reshape((ks, 128) + a.shape[1:]).swapaxes(0, 1))


def _is_act_tile(kt):
    # m-tile kt of a layer's output features: DVE ({0,1}) if even, ACT ({-1,1})
    return kt % 2 == 1


def build_program(bc=BC, bt=BT, dummies=True, reps=1, pipelined=True):
    nbt = bc // bt
    nc = bacc.Bacc("TRN2", target_bir_lowering=False)

    xhi_d = nc.declare_dram_parameter("xhiT", [64, bc], FP16, False)
    xlo_d = nc.declare_dram_parameter("xloT", [64, bc], FP16, False)
    w1t_d = nc.declare_dram_parameter("w1t", [128, 2048], FP16, False)
    w2t_d = nc.declare_dram_parameter("w2t", [128, 16, 1024], FP8, False)
    w3t_d = nc.declare_dram_parameter("w3t", [128, 8, 512], FP8, False)
    w4t_d = nc.declare_dram_parameter("w4t", [128, 4, 64], FP8, False)
    w5t_d = nc.declare_dram_parameter("w5t", [64, 1], BF16, False)
    thr1_d = nc.declare_dram_parameter("thr1", [128, 16], F32, False)
    thr2_d = nc.declare_dram_parameter("thr2", [128, 8], F32, False)
    thr3_d = nc.declare_dram_parameter("thr3", [128, 4], F32, False)
    thr4_d = nc.declare_dram_parameter("thr4", [64, 1], F32, False)
    nthr1_d = nc.declare_dram_parameter("nthr1", [128, 16], F32, False)
    nthr2_d = nc.declare_dram_parameter("nthr2", [128, 8], F32, False)
    nthr3_d = nc.declare_dram_parameter("nthr3", [128, 4], F32, False)
    b5_d = nc.declare_dram_parameter("b5", [1, 1], F32, False)
    out_d = nc.declare_dram_parameter("out", [nbt, bt], F32, True)

    gt = mybir.AluOpType.is_gt
    SIGN = mybir.ActivationFunctionType.Sign

    with tile.TileContext(nc) as tc:
        with ExitStack() as ctx:
            const = ctx.enter_context(tc.tile_pool(name="const", bufs=1))
            xf_p = ctx.enter_context(tc.tile_pool(name="xf", bufs=4))
            h1_p = ctx.enter_context(tc.tile_pool(name="h1", bufs=8))
            h2_p = ctx.enter_context(tc.tile_pool(name="h2", bufs=4))
            h3_p = ctx.enter_context(tc.tile_pool(name="h3", bufs=4))
            h4_p = ctx.enter_context(tc.tile_pool(name="h4", bufs=4))
            o_p = ctx.enter_context(tc.tile_pool(name="o", bufs=nbt))
            ps_mm = ctx.enter_context(tc.tile_pool(name="psmm", bufs=4, space="PSUM"))
            ps_l1 = ps_mm

            ident = const.tile([128, 128], F32)
            make_identity(nc, ident)

            _qs = [nc.sync, nc.scalar, nc.gpsimd]

            def cload(nm, shape, dtype, dram, q=0):
                t = const.tile(shape, dtype, tag=nm, name=nm)
                _qs[q % len(_qs)].dma_start(t[:], dram[:])
                return t

            def front_a(b):
                """xf [128, bt] fp16: rows 0-63 = hi(x)^T, 64-127 = lo(x)^T,
                host-pre-transposed, plain contiguous DMA."""
                xf = xf_p.tile([128, bt], FP16, tag="xf", name="xf")
                nc.sync.dma_start(xf[0:64, :], xhi_d[:, ds(b * bt, bt)])
                nc.sync.dma_start(xf[64:128, :], xlo_d[:, ds(b * bt, bt)])
                return xf

            # startup critical path: xf0 heads the sync queue; w1t spread in
            # 4 chunks across the heads of all three queues (ahead of the 2MB
            # w2t) so L1(tile0)'s m-tiles are all fed by ~1.1us; xf1 preloaded
            # before the w2t sync-part so L1(tile1) is fed by ~1.8us.
            xf0 = front_a(0) if reps == 1 else None
            w1t = const.tile([128, 2048], FP16, tag="w1t", name="w1t")
            nc.scalar.dma_start(w1t[:, 0:512], w1t_d[:, 0:512])
            nc.gpsimd.dma_start(w1t[:, 512:1024], w1t_d[:, 512:1024])
            nc.scalar.dma_start(w1t[:, 1024:1536], w1t_d[:, 1024:1536])
            nc.sync.dma_start(w1t[:, 1536:2048], w1t_d[:, 1536:2048])
            xf1 = front_a(1) if reps == 1 else None
            # w2t split across all three DMA queues so it lands ~3x sooner
            w2t = const.tile([128, 16, 1024], FP8, tag="w2t", name="w2t")
            nc.scalar.dma_start(w2t[:, 0:6, :], w2t_d[:, 0:6, :])
            nc.gpsimd.dma_start(w2t[:, 6:11, :], w2t_d[:, 6:11, :])
            nc.sync.dma_start(w2t[:, 11:16, :], w2t_d[:, 11:16, :])
            w3t = cload("w3t", [128, 8, 512], FP8, w3t_d, q=2)
            w4t = cload("w4t", [128, 4, 64], FP8, w4t_d, q=3)
            w5t = cload("w5t", [64, 1], BF16, w5t_d)
            w5tf = w5t  # alias
            thr1 = cload("thr1", [128, 16], F32, thr1_d, q=3)
            thr2 = cload("thr2", [128, 8], F32, thr2_d, q=2)
            thr3 = cload("thr3", [128, 4], F32, thr3_d)
            thr4 = cload("thr4", [64, 1], F32, thr4_d)
            nthr1 = cload("nthr1", [128, 16], F32, nthr1_d)
            nthr2 = cload("nthr2", [128, 8], F32, nthr2_d)
            nthr3 = cload("nthr3", [128, 4], F32, nthr3_d)
            b5 = cload("b5", [1, 1], F32, b5_d)

            # dummy consumers: absorb const-producing semaphores so
            # steady-state matmuls/epilogues carry at most one wait each.
            # Only L1's consts are absorbed up front; the heavy w2t/w3t loads
            # are absorbed after the first L1 block (emit_late_dummies) so the
            # PE isn't parked on their DMA during startup.
            if dummies:
                dps = ps_mm.tile([128, 128], F32, tag="mm")
                # extra ident matmuls keep the PE busy through the xf0/w1t DMA
                # wait so the HAM activity window ramps to 2.4GHz sooner
                for _ in range(6):
                    nc.tensor.matmul(dps[:], lhsT=ident[:], rhs=ident[:],
                                     start=True, stop=True)
                nc.tensor.matmul(dps[:], lhsT=w1t[:, 0:128], rhs=w1t[:, 0:128],
                                 start=True, stop=True)
                dsb = const.tile([128, 16], F32)
                nc.vector.tensor_copy(dsb[:, 0:16], thr1[:])
                dsb2 = const.tile([128, 16], F32)
                nc.scalar.copy(dsb2[:, 0:16], nthr1[:])

            def emit_late_dummies():
                if not dummies:
                    return
                dps = ps_mm.tile([128, 128], F32, tag="mm")
                nc.tensor.matmul(dps[:], lhsT=w1t[:, 512:640], rhs=w1t[:, 512:640],
                                 start=True, stop=True)
                nc.tensor.matmul(dps[:], lhsT=w1t[:, 1024:1152], rhs=w1t[:, 1024:1152],
                                 start=True, stop=True)
                nc.tensor.matmul(dps[:], lhsT=w1t[:, 1536:1664], rhs=w1t[:, 1536:1664],
                                 start=True, stop=True)
                nc.tensor.matmul(dps[:], lhsT=w2t[:, 0, 0:128], rhs=w2t[:, 0, 0:128],
                                 start=True, stop=True)
                nc.tensor.matmul(dps[:], lhsT=w3t[:, 0, 0:128], rhs=w3t[:, 0, 0:128],
                                 start=True, stop=True)
                nc.tensor.matmul(dps[:64, :64], lhsT=w4t[:, 0, :], rhs=w4t[:, 0, :],
                                 start=True, stop=True)
                nc.tensor.matmul(dps[:1, :1], lhsT=w5t[:], rhs=w5t[:],
                                 start=True, stop=True)
                nc.vector.tensor_copy(dsb[:, 0:8], thr2[:])
                nc.vector.tensor_copy(dsb[:, 0:4], thr3[:])
                nc.vector.tensor_copy(dsb[:64, 0:1], thr4[:])
                nc.scalar.copy(dsb2[:, 0:8], nthr2[:])
                nc.scalar.copy(dsb2[:, 0:4], nthr3[:])
                nc.scalar.copy(dsb2[:1, 0:1], b5[:])

            def epilogue(h_ap, ps_ap, mt, thr, nthr):
                if _is_act_tile(mt):
                    nc.scalar.activation(h_ap, ps_ap, SIGN,
                                         bias=nthr[:, mt : mt + 1], scale=1.0)
                else:
                    nc.vector.tensor_scalar(h_ap, ps_ap, thr[:, mt : mt + 1],
                                            None, gt)

            def l1_emitter(xf):
                """Generator emitting one L1 m-tile (2 matmuls + epilogue) per
                next(); yields (h1a, h1b) half-tiles after all 16."""
                h1a = h1_p.tile([128, 4, 2, bt], FP8, tag="h1", name="h1a")
                h1b = h1_p.tile([128, 4, 2, bt], FP8, tag="h1", name="h1b")

                def emit(mt):
                    ps = ps_l1.tile([128, bt], F32, tag="mm", name="psl1")
                    nc.tensor.matmul(ps[:, 0:512], lhsT=w1t[:, ts(mt, 128)],
                                     rhs=xf[:, 0:512], start=True, stop=True)
                    nc.tensor.matmul(ps[:, 512:1024], lhsT=w1t[:, ts(mt, 128)],
                                     rhs=xf[:, 512:1024], start=True, stop=True)
                    h = h1a if mt < 8 else h1b
                    j = mt % 8
                    epilogue(h[:, j // 2, j % 2, :], ps[:], mt, thr1, nthr1)

                return h1a, h1b, emit

            def mm_layer2_pair(psA, psB, h1abA, h1abB, mt):
                """L2: one weight load streams both tiles of the pair."""
                for k in range(0, 16, 2):
                    lhsT = w2t[:, k : k + 2, ts(mt, 128)]
                    st = k == 0
                    sp = k == 14
                    for ps, h1ab in ((psA, h1abA), (psB, h1abB)):
                        h = h1ab[0] if k < 8 else h1ab[1]
                        kk = k % 8
                        nc.tensor.matmul(ps[:, 0:512], lhsT=lhsT,
                                         rhs=h[:, kk // 2, :, 0:512], perf_mode=DR,
                                         start=st, stop=sp)
                        nc.tensor.matmul(ps[:, 512:1024], lhsT=lhsT,
                                         rhs=h[:, kk // 2, :, 512:1024],
                                         perf_mode=DR, start=st, stop=sp)

            def full_l1(xf):
                h1a, h1b, emit = l1_emitter(xf)
                for mt in range(16):
                    emit(mt)
                return h1a, h1b

            def stage_pair(bA, bB, h1A, h1B, xf_nexts, prev_tails):
                """L2..L5 for batch tiles (bA, bB); each weight block streams
                both tiles. L1 m-tiles of the next pair interleave between L2
                m-tiles; previous pair's L5 tails emitted after the first two
                L2 m-tiles."""
                nxts = []
                emits = []
                for xf_next in xf_nexts:
                    nh1a, nh1b, emit = l1_emitter(xf_next)
                    nxts.append((nh1a, nh1b))
                    emits.append(emit)

                h2 = []
                for nm in ("A", "B"):
                    h2a = h2_p.tile([128, 2, 2, bt], FP8, tag="h2", name="h2a" + nm)
                    h2b = h2_p.tile([128, 2, 2, bt], FP8, tag="h2", name="h2b" + nm)
                    h2.append((h2a, h2b))
                for mt in range(8):
                    psA = ps_mm.tile([128, bt], F32, tag="mm", name="psA")
                    psB = ps_mm.tile([128, bt], F32, tag="mm", name="psB")
                    mm_layer2_pair(psA, psB, h1A, h1B, mt)
                    for i, ps in ((0, psA), (1, psB)):
                        h = h2[i][0] if mt < 4 else h2[i][1]
                        j = mt % 4
                        epilogue(h[:, j // 2, j % 2, :], ps[:], mt, thr2, nthr2)
                    for emit in emits:
                        emit(2 * mt)
                        emit(2 * mt + 1)
                    if mt < len(prev_tails):
                        prev_tails[mt]()

                h3 = []
                for nm in ("A", "B"):
                    h3a = h3_p.tile([128, 1, 2, bt], FP8, tag="h3", name="h3a" + nm)
                    h3b = h3_p.tile([128, 1, 2, bt], FP8, tag="h3", name="h3b" + nm)
                    h3.append((h3a, h3b))
                for mt in range(4):
                    psA = ps_mm.tile([128, bt], F32, tag="mm", name="psA")
                    psB = ps_mm.tile([128, bt], F32, tag="mm", name="psB")
                    for k in range(0, 8, 2):
                        lhsT = w3t[:, k : k + 2, ts(mt, 128)]
                        st = k == 0
                        sp = k == 6
                        for i, ps in ((0, psA), (1, psB)):
                            h = h2[i][0] if k < 4 else h2[i][1]
                            kk = k % 4
                            nc.tensor.matmul(ps[:, 0:512], lhsT=lhsT,
                                             rhs=h[:, kk // 2, :, 0:512],
                                             perf_mode=DR, start=st, stop=sp)
                            nc.tensor.matmul(ps[:, 512:1024], lhsT=lhsT,
                                             rhs=h[:, kk // 2, :, 512:1024],
                                             perf_mode=DR, start=st, stop=sp)
                    for i, ps in ((0, psA), (1, psB)):
                        h = h3[i][0] if mt < 2 else h3[i][1]
                        epilogue(h[:, 0, mt % 2, :], ps[:], mt, thr3, nthr3)

                h4s = []
                ps4A = ps_mm.tile([64, bt], F32, tag="mm", name="ps4A")
                ps4B = ps_mm.tile([64, bt], F32, tag="mm", name="ps4B")
                for i, ps4 in ((0, ps4A), (1, ps4B)):
                    h4 = h4_p.tile([64, bt], BF16, tag="h4", name="h4" + "AB"[i])
                    h4s.append(h4)
                for k in range(0, 4, 2):
                    lhsT = w4t[:, k : k + 2, :]
                    st = k == 0
                    sp = k == 2
                    for i, ps4 in ((0, ps4A), (1, ps4B)):
                        h = h3[i][0] if k < 2 else h3[i][1]
                        nc.tensor.matmul(ps4[:, 0:512], lhsT=lhsT,
                                         rhs=h[:, 0, :, 0:512],
                                         perf_mode=DR, start=st, stop=sp)
                        nc.tensor.matmul(ps4[:, 512:1024], lhsT=lhsT,
                                         rhs=h[:, 0, :, 512:1024],
                                         perf_mode=DR, start=st, stop=sp)
                for i, ps4 in ((0, ps4A), (1, ps4B)):
                    nc.vector.tensor_scalar(h4s[i][:], ps4[:], thr4[:, 0:1],
                                            None, gt)

                def mk_tail(b, h4):
                    def tail():
                        ps5 = ps_l1.tile([1, bt], F32, tag="mm", name="ps5")
                        nc.tensor.matmul(ps5[:, 0:512], lhsT=w5tf[:],
                                         rhs=h4[:, 0:512],
                                         start=True, stop=True)
                        nc.tensor.matmul(ps5[:, 512:1024], lhsT=w5tf[:],
                                         rhs=h4[:, 512:1024], start=True, stop=True)
                        o = o_p.tile([1, bt], F32, tag="o", name="o")
                        nc.scalar.activation(o[:], ps5[:],
                                             mybir.ActivationFunctionType.Sigmoid,
                                             bias=b5[:1, :1], scale=1.0)
                        nc.sync.dma_start(out_d[b : b + 1, :], o[:])
                    return tail

                return nxts, [mk_tail(bA, h4s[0]), mk_tail(bB, h4s[1])]

            rep_ctx = tc.For_i(0, reps, 1) if reps > 1 else None
            if rep_ctx is not None:
                rep_ctx.__enter__()
            h1A = full_l1(xf0 if xf0 is not None else front_a(0))
            emit_late_dummies()
            h1B = full_l1(xf1 if xf1 is not None else front_a(1))
            tails = []
            for p in range(nbt // 2):
                bA, bB = 2 * p, 2 * p + 1
                nxt_fronts = ([front_a(bA + 2), front_a(bB + 2)]
                              if bB + 2 < nbt else [])
                nxts, tails_new = stage_pair(bA, bB, h1A, h1B,
                                             nxt_fronts, tails)
                if nxts:
                    h1A, h1B = nxts
                tails = tails_new
            for t in tails:
                t()
            if rep_ctx is not None:
                rep_ctx.__exit__(None, None, None)

    nc.compile()
    return nc


def prep_weights(w1, b1, w2, b2, w3, b3, w4, b4, w5, b5,
                 *,
                 g1, be1, m1, v1, g2, be2, m2, v2,
                 g3, be3, m3, v3, g4, be4, m4, v4):
    bf = ml_dtypes.bfloat16
    f16 = np.float16
    f8 = ml_dtypes.float8_e4m3
    f64 = np.float64

    # layer 1: sign weights duplicated on both K-halves (2-term fp16 split)
    w1b = np.sign(w1).astype(f64)                                       # [2048,64]
    thr1 = _thr(b1, g1, be1, m1, v1)                                    # [2048]

    def scaled(wb, thr_next, n_in):
        """Scale ACT-coded input columns by 1/2 and fold the matching
        -0.5*sum(sign) correction into the next layer's threshold."""
        wb = wb.copy()
        corr = np.zeros(wb.shape[0], f64)
        for kt in range(n_in // 128):
            if _is_act_tile(kt):
                cols = slice(kt * 128, (kt + 1) * 128)
                corr += wb[:, cols].sum(axis=1) * 0.5
                wb[:, cols] *= 0.5
        return wb, thr_next - corr

    w2b, thr2 = scaled(np.sign(w2).astype(f64), _thr(b2, g2, be2, m2, v2), 2048)
    w3b, thr3 = scaled(np.sign(w3).astype(f64), _thr(b3, g3, be3, m3, v3), 1024)
    w4b, thr4 = scaled(np.sign(w4).astype(f64), _thr(b4, g4, be4, m4, v4), 512)

    out = dict(
        w1t=np.ascontiguousarray(
            np.concatenate([w1b.T, w1b.T], axis=0).astype(f16)),        # [128,2048]
        w2t=_feat_major(w2b.T.astype(f8), 2048),                        # [128,16,1024]
        w3t=_feat_major(w3b.T.astype(f8), 1024),                        # [128,8,512]
        w4t=_feat_major(w4b.T.astype(f8), 512),                         # [128,4,64]
        w5t=np.ascontiguousarray(w5.reshape(64, 1).astype(bf)),         # [64,1]
        thr1=_feat_major(thr1.astype(np.float32), 2048),
        thr2=_feat_major(thr2.astype(np.float32), 1024),
        thr3=_feat_major(thr3.astype(np.float32), 512),
        thr4=np.ascontiguousarray(thr4.astype(np.float32).reshape(64, 1)),
        nthr1=_feat_major((-thr1).astype(np.float32), 2048),
        nthr2=_feat_major((-thr2).astype(np.float32), 1024),
        nthr3=_feat_major((-thr3).astype(np.float32), 512),
        b5=np.asarray(b5, np.float32).reshape(1, 1),
    )
    return out


def make_in_maps(inputs):
    x = np.asarray(inputs["x"], np.float32)
    xhi = x.astype(np.float16)
    xlo = (x - xhi.astype(np.float32)).astype(np.float16)
    wmap = prep_weights(**{k: np.asarray(v) for k, v in inputs.items() if k != "x"})
    in_maps = []
    for c in range(N_CORES):
        m = dict(wmap)
        m["xhiT"] = np.ascontiguousarray(xhi[c * BC : (c + 1) * BC].T)
        m["xloT"] = np.ascontiguousarray(xlo[c * BC : (c + 1) * BC].T)
        in_maps.append(m)
    return in_maps


_CACHED = {}


def run(inputs, trace=False):
    if "nc" not in _CACHED:
        _CACHED["nc"] = build_program()
    nc = _CACHED["nc"]

    in_maps = make_in_maps(inputs)

    res = run_bass_kernel_spmd(nc, in_maps, list(range(N_CORES)), trace=trace)
    out = np.concatenate(
        [np.asarray(r["out"]).reshape(BC, 1) for r in res.results], axis=0
    )
    return out, res


def kernel(**inputs) -> np.ndarray:
    out, _ = run(inputs, trace=False)
    return out

